# revision 14
# baseline (speedup 1.0000x reference)
"""CrossAttend Trainium2 kernel: 8-way data-parallel over batch.

Full inputs arrive here; we shard batch B=16 across 8 NeuronCores
(2 batch elements per core), replicate the 512x512 projection weights,
run one SPMD Bass/Tile kernel, and concatenate the per-core outputs.

Math notes (validated against the fp32 reference):
  - bk drops out entirely: it shifts every sim row by a constant per q,
    and softmax over k is shift-invariant.
  - qpk := qp @ Wk is shared by both attentions:
        sim  = qpk @ qp.T   (+ per-q const),   sim2 = qpk @ opp.T (+ const)
  - softmax is computed without max-subtraction (logits are O(5), exp is
    safe); the self-attention diagonal is zeroed after exp.
  - rowsums are FUSED into the AV matmul: v tiles carry a ones column at
    free offset 256 (layout [256 h | 1 | 256 h | 1] = 514 wide), and each
    AV output tile is computed as two N=257 matmuls into two PSUM banks.
    Column 256 of bank A accumulates the softmax denominator for free --
    no separate N=4 rowsum matmuls (saves ~26ns x 256 extra-instruction
    issues per core vs the old scheme).

All matmuls run in fp16 (PSUM accumulation stays fp32): fp16 streams
1 col/cycle @2.4GHz like bf16 but carries 10 mantissa bits, dropping the
end-to-end rel err ~7x (1.1e-3 vs 7.9e-3). q and opp are pre-transposed
and cast on the host, so the kernel does no PE transposes at all.

On-chip layouts per batch element:
  qT, qpT, qpkT, oppT : [128, 4, 1024]  (h on partitions)
  v, opp_v            : [128, 8, 514]   (l on partitions, ones at 256/513)
  PexpT               : [128, 8, 1024]  (k on partitions, q free)

Startup: input DMAs for batch 0 are split across the sync/vector/scalar
rings (and weights on gpsimd, in first-use-sized chunks) so the first
real matmul can issue ~7.5us in; 3 dummy warmup matmuls (no DMA deps)
cover the DMA ramp and start the HAM clock warmup immediately. Batch 1
inputs are prefetched right behind batch 0's on the same rings.

Engine budget: TensorE streams matmuls back-to-back (the bottleneck);
ScalarE does PSUM evacuations that need bias/exp/out-scale; VectorE does
the remaining evacuations and reciprocals; input/b1 DMAs ride sync +
vector + scalar rings, weights ride gpsimd, outputs ride sync.
"""

import contextlib
import math

import ml_dtypes
import numpy as np

import concourse.bass as bass
import concourse.mybir as mybir
import concourse.tile as tile
from concourse import bacc
from concourse.bass_utils import run_bass_kernel_spmd

F32 = mybir.dt.float32
F16 = mybir.dt.float16

B = 16
H = 512
L = 1024
P = 128
NCORES = 8
BPC = B // NCORES   # batch elements per core
HT = H // P         # 4 h-tiles
LT = L // P         # 8 l-tiles
QC = L // 512       # 2 q-chunks of 512
VW = 514            # v tile width: [256 h | ones | 256 h | ones]
SCALE = 1.0 / math.sqrt(H)


def _build_core_kernel(ctx, tc, ins, outs):
    nc = tc.nc
    AF = mybir.ActivationFunctionType

    qT_d = ins["qT"]        # [BPC, H, L] f16 (host pre-transposed)
    oppT_d = ins["oppT"]    # [BPC, H, L] f16
    self_d = outs["self_out"]
    oout_d = outs["opp_out"]

    wpool = ctx.enter_context(tc.tile_pool(name="w", bufs=1))
    big = ctx.enter_context(tc.tile_pool(name="big", bufs=6))
    vpool = ctx.enter_context(tc.tile_pool(name="v", bufs=2))
    ppool = ctx.enter_context(tc.tile_pool(name="P", bufs=2))
    opool = ctx.enter_context(tc.tile_pool(name="o", bufs=4))
    rpool = ctx.enter_context(tc.tile_pool(name="r", bufs=4))
    pspool = ctx.enter_context(tc.tile_pool(name="ps", bufs=8, space="PSUM"))

    # --- PE warmup: dummy matmuls whose only dep is a tiny gpsimd memset,
    # emitted first so they hold the earliest PE queue slots.  They keep the
    # PE busy through the input-DMA ramp and start the ~3.4us HAM clock-gate
    # warmup (to K=8/8, 2.4GHz) immediately.  Results go to a PSUM slice
    # nobody reads.
    warm = wpool.tile([P, P], F16, tag="warm")
    nc.gpsimd.memset(warm[:], 0.0)
    pw = pspool.tile([P, 512], F32, tag="ps")
    NWARM = 32
    for i in range(NWARM):
        nc.tensor.matmul(pw[:, 0:P], lhsT=warm[:], rhs=warm[:],
                         start=(i == 0), stop=(i == NWARM - 1))

    # --- DMA plan.  Per-ring bandwidth during the 8-core ramp is only
    # ~55-60GB/s, and a blocked dma_start instruction stalls its host
    # engine's whole FIFO, so: the scalar(ACT) ring carries ONLY the tiny
    # bq vector (ACT must stay free for PSUM evacuation), and batch-0
    # inputs + weights are split between the sync and gpsimd rings in
    # chunks sized and ordered by exact first-use (qc-outer projection
    # consumes qT l-half 0 for groups 0-3, wq col-block ht for group ht).
    bq = wpool.tile([P, HT], F32, tag="bq")

    wq = wpool.tile([P, HT, H], F16, tag="wq")
    wk = wpool.tile([P, HT, H], F16, tag="wk")
    wv = wpool.tile([P, HT, H], F16, tag="wv")
    bvb = wpool.tile([P, H], F16, tag="bvb")
    wqr = ins["WqT"].rearrange("(ko ki) m -> ki ko m", ki=P)
    wkr = ins["Wk"].rearrange("(ko ki) m -> ki ko m", ki=P)
    wvr = ins["WvT"].rearrange("(ko ki) m -> ki ko m", ki=P)

    qT0 = big.tile([P, HT, L], F16, tag="big")
    oppT0 = big.tile([P, HT, L], F16, tag="big")
    qT1 = big.tile([P, HT, L], F16, tag="big")
    oppT1 = big.tile([P, HT, L], F16, tag="big")
    q0r = qT_d[0].rearrange("(ko ki) l -> ki ko l", ki=P)
    o0r = oppT_d[0].rearrange("(ko ki) l -> ki ko l", ki=P)
    q1r = qT_d[1].rearrange("(ko ki) l -> ki ko l", ki=P)
    o1r = oppT_d[1].rearrange("(ko ki) l -> ki ko l", ki=P)

    # scalar(ACT) ring (HW-DGE, no teardown cost): early wait-free chunks
    # only -- all issued well before ACT's first PSUM evacuation (~13us).
    nc.scalar.dma_start(qT0[:, 1:2, 0:512], q0r[:, 1:2, 0:512])
    nc.scalar.dma_start(qT0[:, 2:3, 0:512], q0r[:, 2:3, 0:512])
    nc.scalar.dma_start(oppT0[:, :, 0:512], o0r[:, :, 0:512])
    nc.scalar.dma_start(oppT0[:, :, 512:L], o0r[:, :, 512:L])

    # sync ring (HW-DGE): qp/qpk-phase chunks in consumption order, then
    # batch-1 prefetch; output tiles ride this ring later.
    nc.sync.dma_start(bq[:], ins["bq_p"][:])
    nc.sync.dma_start(wq[:, :, 0:P], wqr[:, :, 0:P])
    nc.sync.dma_start(qT0[:, 0:1, 0:512], q0r[:, 0:1, 0:512])
    nc.sync.dma_start(wq[:, :, P:2 * P], wqr[:, :, P:2 * P])
    nc.sync.dma_start(qT0[:, 0:2, 512:L], q0r[:, 0:2, 512:L])
    nc.sync.dma_start(wk[:, :, 0:P], wkr[:, :, 0:P])
    nc.sync.dma_start(wk[:, :, P:2 * P], wkr[:, :, P:2 * P])
    nc.sync.dma_start(qT1[:, :, 0:512], q1r[:, :, 0:512])
    nc.sync.dma_start(qT1[:, :, 512:L], q1r[:, :, 512:L])
    nc.sync.dma_start(oppT1[:, :, 0:512], o1r[:, :, 0:512])
    nc.sync.dma_start(oppT1[:, :, 512:L], o1r[:, :, 512:L])

    # gpsimd ring (SW-DGE: each dma_start costs ~100ns of teardown, keep
    # the count low): remaining ramp chunks + late weights.
    nc.gpsimd.dma_start(qT0[:, 3:4, 0:512], q0r[:, 3:4, 0:512])
    nc.gpsimd.dma_start(wq[:, :, 2 * P:3 * P], wqr[:, :, 2 * P:3 * P])
    nc.gpsimd.dma_start(wq[:, :, 3 * P:H], wqr[:, :, 3 * P:H])
    nc.gpsimd.dma_start(qT0[:, 2:4, 512:L], q0r[:, 2:4, 512:L])
    nc.gpsimd.dma_start(wk[:, :, 2 * P:3 * P], wkr[:, :, 2 * P:3 * P])
    nc.gpsimd.dma_start(wk[:, :, 3 * P:H], wkr[:, :, 3 * P:H])
    nc.gpsimd.dma_start(wv[:], wvr[:])
    nc.gpsimd.dma_start(bvb[:], ins["bv_b"][:])

    qTs, oppTs = (qT0, qT1), (oppT0, oppT1)

    def proj_T(src_T, w, bias=None):
        """dst[h_out-part, l] = sum_hin w[hin, hout-tile].T @ src_T[hin, l].
        qc-outer: groups 0-3 consume only src l-half 0 + one w col-block
        each, matching the DMA chunk arrival order."""
        dst = big.tile([P, HT, L], F16, tag="big")
        for qc in range(QC):
            for ht in range(HT):
                ps = pspool.tile([P, 512], F32, tag="ps")
                for hc in range(HT):
                    nc.tensor.matmul(
                        ps[:],
                        lhsT=w[:, hc, P * ht:P * (ht + 1)],
                        rhs=src_T[:, hc, 512 * qc:512 * (qc + 1)],
                        start=(hc == 0),
                        stop=(hc == HT - 1),
                    )
                d = dst[:, ht, 512 * qc:512 * (qc + 1)]
                if bias is not None:
                    nc.scalar.activation(d, ps[:], AF.Identity,
                                         bias=bias[:, ht:ht + 1], scale=1.0)
                else:
                    nc.vector.tensor_copy(d, ps[:])
        return dst

    def proj_nat(src_T, w_rhs, bias_b):
        """dst[l-part, h_out] = src_T[hin, l-tile].T @ w_rhs[hin, hout]
        + bias, written in the [256 | 1 | 256 | 1] AV layout with ones
        columns at free offsets 256 and 513."""
        dst = vpool.tile([P, LT, VW], F16, tag="v")
        nc.vector.memset(dst[:, :, 256:257], 1.0)
        nc.vector.memset(dst[:, :, 513:514], 1.0)
        for lt in range(LT):
            ps = pspool.tile([P, 512], F32, tag="ps")
            for hc in range(HT):
                nc.tensor.matmul(
                    ps[:],
                    lhsT=src_T[:, hc, P * lt:P * (lt + 1)],
                    rhs=w_rhs[:, hc, :],
                    start=(hc == 0),
                    stop=(hc == HT - 1),
                )
            nc.vector.tensor_tensor(dst[:, lt, 0:256], ps[:, 0:256],
                                    bias_b[:, 0:256], mybir.AluOpType.add)
            nc.vector.tensor_tensor(dst[:, lt, 257:513], ps[:, 256:512],
                                    bias_b[:, 256:512], mybir.AluOpType.add)
        return dst

    def scores(lhsT_T, qpkT, masked):
        """PexpT[k, q] = exp(scale * lhsT_T.T @ qpkT)."""
        pexp = ppool.tile([P, LT, L], F16, tag="P")
        for qc in range(QC):
            for ko in range(LT):
                ps = pspool.tile([P, 512], F32, tag="ps")
                for hc in range(HT):
                    nc.tensor.matmul(
                        ps[:],
                        lhsT=lhsT_T[:, hc, P * ko:P * (ko + 1)],
                        rhs=qpkT[:, hc, 512 * qc:512 * (qc + 1)],
                        start=(hc == 0),
                        stop=(hc == HT - 1),
                    )
                d = pexp[:, ko, 512 * qc:512 * (qc + 1)]
                nc.scalar.activation(d, ps[:], AF.Exp, scale=SCALE)
                if masked and qc == ko // (512 // P):
                    m = ko % (512 // P)
                    nc.gpsimd.affine_select(
                        out=d, in_=d,
                        compare_op=mybir.AluOpType.not_equal,
                        fill=0.0, base=P * m,
                        pattern=[[-1, 512]], channel_multiplier=1,
                    )
        return pexp

    def attn_av(pexp, vv, out_d, b):
        """out = (PexpT.T @ v) / rowsum; the rowsum accumulates in column
        256 of PSUM bank A via the ones column riding the v tiles."""
        for qo in range(LT):
            psA = pspool.tile([P, 512], F32, tag="ps")
            psB = pspool.tile([P, 512], F32, tag="ps")
            for ko in range(LT):
                lt = pexp[:, ko, P * qo:P * (qo + 1)]
                nc.tensor.matmul(psA[:, 0:257], lhsT=lt, rhs=vv[:, ko, 0:257],
                                 start=(ko == 0), stop=(ko == LT - 1))
                nc.tensor.matmul(psB[:, 0:257], lhsT=lt, rhs=vv[:, ko, 257:VW],
                                 start=(ko == 0), stop=(ko == LT - 1))
            rc = rpool.tile([P, 1], F32, tag="r")
            nc.vector.reciprocal(rc[:], psA[:, 256:257])
            ot = opool.tile([P, 512], F16, tag="o")
            nc.scalar.activation(ot[:, 0:256], psA[:, 0:256], AF.Copy,
                                 scale=rc[:, 0:1])
            nc.scalar.activation(ot[:, 256:512], psB[:, 0:256], AF.Copy,
                                 scale=rc[:, 0:1])
            nc.sync.dma_start(out_d[b, P * qo:P * (qo + 1), :], ot[:])

    for b in range(BPC):
        qpT = proj_T(qTs[b], wq, bias=bq)
        qpkT = proj_T(qpT, wk)
        pexp1 = scores(qpT, qpkT, masked=True)
        vv = proj_nat(qpT, wv, bvb)
        ovv = proj_nat(oppTs[b], wv, bvb)
        pexp2 = scores(oppTs[b], qpkT, masked=False)
        attn_av(pexp1, vv, self_d, b)
        attn_av(pexp2, ovv, oout_d, b)


_NC_CACHE = None


def _get_module():
    global _NC_CACHE
    if _NC_CACHE is not None:
        return _NC_CACHE
    nc = bacc.Bacc(None, target_bir_lowering=False, debug=False)
    f32 = mybir.dt.float32
    f16 = mybir.dt.float16
    ins = {
        "qT": nc.dram_tensor("qT", [BPC, H, L], f16, kind="ExternalInput").ap(),
        "oppT": nc.dram_tensor("oppT", [BPC, H, L], f16,
                               kind="ExternalInput").ap(),
        "WqT": nc.dram_tensor("WqT", [H, H], f16, kind="ExternalInput").ap(),
        "Wk": nc.dram_tensor("Wk", [H, H], f16, kind="ExternalInput").ap(),
        "WvT": nc.dram_tensor("WvT", [H, H], f16, kind="ExternalInput").ap(),
        "bq_p": nc.dram_tensor("bq_p", [P, HT], f32, kind="ExternalInput").ap(),
        "bv_b": nc.dram_tensor("bv_b", [P, H], f16, kind="ExternalInput").ap(),
    }
    outs = {
        "self_out": nc.dram_tensor("self_out", [BPC, L, H], f16,
                                   kind="ExternalOutput").ap(),
        "opp_out": nc.dram_tensor("opp_out", [BPC, L, H], f16,
                                  kind="ExternalOutput").ap(),
    }
    with tile.TileContext(nc) as tc:
        with contextlib.ExitStack() as ctx:
            _build_core_kernel(ctx, tc, ins, outs)
    nc.compile()
    _NC_CACHE = nc
    return nc


def kernel(q, opp, Wq, bq, Wk, bk, Wv, bv):
    f16 = np.float16
    qT = np.ascontiguousarray(
        np.asarray(q, dtype=np.float32).astype(f16).transpose(0, 2, 1))
    oppT = np.ascontiguousarray(
        np.asarray(opp, dtype=np.float32).astype(f16).transpose(0, 2, 1))
    Wq = np.asarray(Wq, dtype=np.float32)
    Wk = np.asarray(Wk, dtype=np.float32)
    Wv = np.asarray(Wv, dtype=np.float32)
    bq = np.asarray(bq, dtype=np.float32)
    bv = np.asarray(bv, dtype=np.float32)
    # bk is mathematically irrelevant (softmax shift-invariance); unused.

    shared = {
        "WqT": np.ascontiguousarray(Wq.T.astype(f16)),
        "Wk": np.ascontiguousarray(Wk.astype(f16)),
        "WvT": np.ascontiguousarray(Wv.T.astype(f16)),
        "bq_p": np.ascontiguousarray(bq.reshape(HT, P).T),
        "bv_b": np.ascontiguousarray(np.tile(bv, (P, 1)).astype(f16)),
    }
    in_maps = []
    for c in range(NCORES):
        sl = slice(c * BPC, (c + 1) * BPC)
        in_maps.append({
            "qT": np.ascontiguousarray(qT[sl]),
            "oppT": np.ascontiguousarray(oppT[sl]),
            **shared,
        })

    nc = _get_module()
    res = run_bass_kernel_spmd(nc, in_maps, core_ids=list(range(NCORES)))
    self_out = np.concatenate(
        [r["self_out"].astype(np.float32) for r in res.results], axis=0)
    opp_out = np.concatenate(
        [r["opp_out"].astype(np.float32) for r in res.results], axis=0)
    return (self_out, opp_out)


# revision 15
# speedup vs baseline: 1.0268x; 1.0268x over previous
"""CrossAttend Trainium2 kernel: 8-way data-parallel over batch.

Full inputs arrive here; we shard batch B=16 across 8 NeuronCores
(2 batch elements per core), replicate the 512x512 projection weights,
run one SPMD Bass/Tile kernel, and concatenate the per-core outputs.

Math notes (validated against the fp32 reference):
  - bk drops out entirely: it shifts every sim row by a constant per q,
    and softmax over k is shift-invariant.
  - qpk := qp @ Wk is shared by both attentions:
        sim  = qpk @ qp.T   (+ per-q const),   sim2 = qpk @ opp.T (+ const)
  - softmax is computed without max-subtraction (logits are O(5), exp is
    safe); the self-attention diagonal is zeroed after exp.
  - rowsums are FUSED into the AV matmul: v tiles carry a ones column at
    free offset 256 (layout [256 h | 1 | 256 h | 1] = 514 wide), and each
    AV output tile is computed as two N=257 matmuls into two PSUM banks.
    Column 256 of bank A accumulates the softmax denominator for free --
    no separate N=4 rowsum matmuls (saves ~26ns x 256 extra-instruction
    issues per core vs the old scheme).

All matmuls run in fp16 (PSUM accumulation stays fp32): fp16 streams
1 col/cycle @2.4GHz like bf16 but carries 10 mantissa bits, dropping the
end-to-end rel err ~7x (1.1e-3 vs 7.9e-3). q and opp are pre-transposed
and cast on the host, so the kernel does no PE transposes at all.

On-chip layouts per batch element:
  qT, qpT, qpkT, oppT : [128, 4, 1024]  (h on partitions)
  v, opp_v            : [128, 8, 514]   (l on partitions, ones at 256/513)
  PexpT               : [128, 8, 1024]  (k on partitions, q free)

Startup: input DMAs for batch 0 are split across the sync/vector/scalar
rings (and weights on gpsimd, in first-use-sized chunks) so the first
real matmul can issue ~7.5us in; 3 dummy warmup matmuls (no DMA deps)
cover the DMA ramp and start the HAM clock warmup immediately. Batch 1
inputs are prefetched right behind batch 0's on the same rings.

Engine budget: TensorE streams matmuls back-to-back (the bottleneck);
ScalarE does PSUM evacuations that need bias/exp/out-scale; VectorE does
the remaining evacuations and reciprocals; input/b1 DMAs ride sync +
vector + scalar rings, weights ride gpsimd, outputs ride sync.
"""

import contextlib
import math

import ml_dtypes
import numpy as np

import concourse.bass as bass
import concourse.mybir as mybir
import concourse.tile as tile
from concourse import bacc
from concourse.bass_utils import run_bass_kernel_spmd

F32 = mybir.dt.float32
F16 = mybir.dt.float16

B = 16
H = 512
L = 1024
P = 128
NCORES = 8
BPC = B // NCORES   # batch elements per core
HT = H // P         # 4 h-tiles
LT = L // P         # 8 l-tiles
QC = L // 512       # 2 q-chunks of 512
VW = 514            # v tile width: [256 h | ones | 256 h | ones]
SCALE = 1.0 / math.sqrt(H)


def _build_core_kernel(ctx, tc, ins, outs):
    nc = tc.nc
    AF = mybir.ActivationFunctionType

    qT_d = ins["qT"]        # [BPC, H, L] f16 (host pre-transposed)
    oppT_d = ins["oppT"]    # [BPC, H, L] f16
    self_d = outs["self_out"]
    oout_d = outs["opp_out"]

    wpool = ctx.enter_context(tc.tile_pool(name="w", bufs=1))
    big = ctx.enter_context(tc.tile_pool(name="big", bufs=6))
    vpool = ctx.enter_context(tc.tile_pool(name="v", bufs=2))
    ppool = ctx.enter_context(tc.tile_pool(name="P", bufs=2))
    opool = ctx.enter_context(tc.tile_pool(name="o", bufs=4))
    rpool = ctx.enter_context(tc.tile_pool(name="r", bufs=4))
    pspool = ctx.enter_context(tc.tile_pool(name="ps", bufs=8, space="PSUM"))

    # --- PE warmup: dummy matmuls whose only dep is a tiny gpsimd memset,
    # emitted first so they hold the earliest PE queue slots.  They keep the
    # PE busy through the input-DMA ramp and start the ~3.4us HAM clock-gate
    # warmup (to K=8/8, 2.4GHz) immediately.  Results go to a PSUM slice
    # nobody reads.
    warm = wpool.tile([P, P], F16, tag="warm")
    nc.gpsimd.memset(warm[:], 0.0)
    pw = pspool.tile([P, 512], F32, tag="ps")
    NWARM = 28
    for i in range(NWARM):
        nc.tensor.matmul(pw[:, 0:P], lhsT=warm[:], rhs=warm[:],
                         start=(i == 0), stop=(i == NWARM - 1))

    # --- DMA plan.  Per-ring bandwidth during the 8-core ramp is only
    # ~55-60GB/s, and a blocked dma_start instruction stalls its host
    # engine's whole FIFO, so: the scalar(ACT) ring carries ONLY the tiny
    # bq vector (ACT must stay free for PSUM evacuation), and batch-0
    # inputs + weights are split between the sync and gpsimd rings in
    # chunks sized and ordered by exact first-use (qc-outer projection
    # consumes qT l-half 0 for groups 0-3, wq col-block ht for group ht).
    bq = wpool.tile([P, HT], F32, tag="bq")

    wq = wpool.tile([P, HT, H], F16, tag="wq")
    wk = wpool.tile([P, HT, H], F16, tag="wk")
    wv = wpool.tile([P, HT, H], F16, tag="wv")
    bvb = wpool.tile([P, H], F16, tag="bvb")
    wqr = ins["WqT"].rearrange("(ko ki) m -> ki ko m", ki=P)
    wkr = ins["Wk"].rearrange("(ko ki) m -> ki ko m", ki=P)
    wvr = ins["WvT"].rearrange("(ko ki) m -> ki ko m", ki=P)

    qT0 = big.tile([P, HT, L], F16, tag="big")
    oppT0 = big.tile([P, HT, L], F16, tag="big")
    qT1 = big.tile([P, HT, L], F16, tag="big")
    oppT1 = big.tile([P, HT, L], F16, tag="big")
    q0r = qT_d[0].rearrange("(ko ki) l -> ki ko l", ki=P)
    o0r = oppT_d[0].rearrange("(ko ki) l -> ki ko l", ki=P)
    q1r = qT_d[1].rearrange("(ko ki) l -> ki ko l", ki=P)
    o1r = oppT_d[1].rearrange("(ko ki) l -> ki ko l", ki=P)

    # Ring plan: ~100GB/s per ring + ~2us latency during the 8-core ramp;
    # round-robin the group-0-critical chunks across all three rings in
    # consumption order.  scalar(ACT) ring is HW-DGE with no teardown cost
    # but every dma_start occupies the ACT FIFO ~0.7us, so all its issues
    # must land before ACT's first PSUM evacuation (~13us).  gpsimd is
    # SW-DGE (each dma_start costs ~100ns of teardown): keep its count low.
    nc.scalar.dma_start(qT0[:, 1:2, 0:512], q0r[:, 1:2, 0:512])
    nc.scalar.dma_start(qT0[:, 3:4, 0:512], q0r[:, 3:4, 0:512])
    nc.scalar.dma_start(wq[:, :, 2 * P:3 * P], wqr[:, :, 2 * P:3 * P])
    nc.scalar.dma_start(wk[:, :, P:2 * P], wkr[:, :, P:2 * P])
    nc.scalar.dma_start(wk[:, :, 3 * P:H], wkr[:, :, 3 * P:H])
    nc.scalar.dma_start(oppT0[:, :, 0:512], o0r[:, :, 0:512])
    nc.scalar.dma_start(oppT0[:, :, 512:L], o0r[:, :, 512:L])

    nc.sync.dma_start(bq[:], ins["bq_p"][:])
    nc.sync.dma_start(qT0[:, 0:1, 0:512], q0r[:, 0:1, 0:512])
    nc.sync.dma_start(wq[:, :, P:2 * P], wqr[:, :, P:2 * P])
    nc.sync.dma_start(qT0[:, 0:2, 512:L], q0r[:, 0:2, 512:L])
    nc.sync.dma_start(wk[:, :, 0:P], wkr[:, :, 0:P])
    nc.sync.dma_start(qT1[:, :, 0:512], q1r[:, :, 0:512])
    nc.sync.dma_start(qT1[:, :, 512:L], q1r[:, :, 512:L])
    nc.sync.dma_start(oppT1[:, :, 0:512], o1r[:, :, 0:512])
    nc.sync.dma_start(oppT1[:, :, 512:L], o1r[:, :, 512:L])

    nc.gpsimd.dma_start(wq[:, :, 0:P], wqr[:, :, 0:P])
    nc.gpsimd.dma_start(qT0[:, 2:3, 0:512], q0r[:, 2:3, 0:512])
    nc.gpsimd.dma_start(wq[:, :, 3 * P:H], wqr[:, :, 3 * P:H])
    nc.gpsimd.dma_start(qT0[:, 2:4, 512:L], q0r[:, 2:4, 512:L])
    nc.gpsimd.dma_start(wk[:, :, 2 * P:3 * P], wkr[:, :, 2 * P:3 * P])
    nc.gpsimd.dma_start(wv[:], wvr[:])
    nc.gpsimd.dma_start(bvb[:], ins["bv_b"][:])

    qTs, oppTs = (qT0, qT1), (oppT0, oppT1)

    def proj_T(src_T, w, bias=None):
        """dst[h_out-part, l] = sum_hin w[hin, hout-tile].T @ src_T[hin, l].
        qc-outer: groups 0-3 consume only src l-half 0 + one w col-block
        each, matching the DMA chunk arrival order."""
        dst = big.tile([P, HT, L], F16, tag="big")
        for qc in range(QC):
            for ht in range(HT):
                ps = pspool.tile([P, 512], F32, tag="ps")
                for hc in range(HT):
                    nc.tensor.matmul(
                        ps[:],
                        lhsT=w[:, hc, P * ht:P * (ht + 1)],
                        rhs=src_T[:, hc, 512 * qc:512 * (qc + 1)],
                        start=(hc == 0),
                        stop=(hc == HT - 1),
                    )
                d = dst[:, ht, 512 * qc:512 * (qc + 1)]
                if bias is not None:
                    nc.scalar.activation(d, ps[:], AF.Identity,
                                         bias=bias[:, ht:ht + 1], scale=1.0)
                else:
                    nc.vector.tensor_copy(d, ps[:])
        return dst

    def proj_nat(src_T, w_rhs, bias_b):
        """dst[l-part, h_out] = src_T[hin, l-tile].T @ w_rhs[hin, hout]
        + bias, written in the [256 | 1 | 256 | 1] AV layout with ones
        columns at free offsets 256 and 513."""
        dst = vpool.tile([P, LT, VW], F16, tag="v")
        nc.vector.memset(dst[:, :, 256:257], 1.0)
        nc.vector.memset(dst[:, :, 513:514], 1.0)
        for lt in range(LT):
            ps = pspool.tile([P, 512], F32, tag="ps")
            for hc in range(HT):
                nc.tensor.matmul(
                    ps[:],
                    lhsT=src_T[:, hc, P * lt:P * (lt + 1)],
                    rhs=w_rhs[:, hc, :],
                    start=(hc == 0),
                    stop=(hc == HT - 1),
                )
            nc.vector.tensor_tensor(dst[:, lt, 0:256], ps[:, 0:256],
                                    bias_b[:, 0:256], mybir.AluOpType.add)
            nc.vector.tensor_tensor(dst[:, lt, 257:513], ps[:, 256:512],
                                    bias_b[:, 256:512], mybir.AluOpType.add)
        return dst

    def scores(lhsT_T, qpkT, masked):
        """PexpT[k, q] = exp(scale * lhsT_T.T @ qpkT)."""
        pexp = ppool.tile([P, LT, L], F16, tag="P")
        for qc in range(QC):
            for ko in range(LT):
                ps = pspool.tile([P, 512], F32, tag="ps")
                for hc in range(HT):
                    nc.tensor.matmul(
                        ps[:],
                        lhsT=lhsT_T[:, hc, P * ko:P * (ko + 1)],
                        rhs=qpkT[:, hc, 512 * qc:512 * (qc + 1)],
                        start=(hc == 0),
                        stop=(hc == HT - 1),
                    )
                d = pexp[:, ko, 512 * qc:512 * (qc + 1)]
                nc.scalar.activation(d, ps[:], AF.Exp, scale=SCALE)
                if masked and qc == ko // (512 // P):
                    m = ko % (512 // P)
                    nc.gpsimd.affine_select(
                        out=d, in_=d,
                        compare_op=mybir.AluOpType.not_equal,
                        fill=0.0, base=P * m,
                        pattern=[[-1, 512]], channel_multiplier=1,
                    )
        return pexp

    def attn_av(pexp, vv, out_d, b):
        """out = (PexpT.T @ v) / rowsum; the rowsum accumulates in column
        256 of PSUM bank A via the ones column riding the v tiles."""
        for qo in range(LT):
            psA = pspool.tile([P, 512], F32, tag="ps")
            psB = pspool.tile([P, 512], F32, tag="ps")
            for ko in range(LT):
                lt = pexp[:, ko, P * qo:P * (qo + 1)]
                nc.tensor.matmul(psA[:, 0:257], lhsT=lt, rhs=vv[:, ko, 0:257],
                                 start=(ko == 0), stop=(ko == LT - 1))
                nc.tensor.matmul(psB[:, 0:257], lhsT=lt, rhs=vv[:, ko, 257:VW],
                                 start=(ko == 0), stop=(ko == LT - 1))
            rc = rpool.tile([P, 1], F32, tag="r")
            nc.vector.reciprocal(rc[:], psA[:, 256:257])
            ot = opool.tile([P, 512], F16, tag="o")
            nc.scalar.activation(ot[:, 0:256], psA[:, 0:256], AF.Copy,
                                 scale=rc[:, 0:1])
            nc.scalar.activation(ot[:, 256:512], psB[:, 0:256], AF.Copy,
                                 scale=rc[:, 0:1])
            nc.sync.dma_start(out_d[b, P * qo:P * (qo + 1), :], ot[:])

    for b in range(BPC):
        qpT = proj_T(qTs[b], wq, bias=bq)
        qpkT = proj_T(qpT, wk)
        pexp1 = scores(qpT, qpkT, masked=True)
        vv = proj_nat(qpT, wv, bvb)
        ovv = proj_nat(oppTs[b], wv, bvb)
        pexp2 = scores(oppTs[b], qpkT, masked=False)
        attn_av(pexp1, vv, self_d, b)
        attn_av(pexp2, ovv, oout_d, b)


_NC_CACHE = None


def _get_module():
    global _NC_CACHE
    if _NC_CACHE is not None:
        return _NC_CACHE
    nc = bacc.Bacc(None, target_bir_lowering=False, debug=False)
    f32 = mybir.dt.float32
    f16 = mybir.dt.float16
    ins = {
        "qT": nc.dram_tensor("qT", [BPC, H, L], f16, kind="ExternalInput").ap(),
        "oppT": nc.dram_tensor("oppT", [BPC, H, L], f16,
                               kind="ExternalInput").ap(),
        "WqT": nc.dram_tensor("WqT", [H, H], f16, kind="ExternalInput").ap(),
        "Wk": nc.dram_tensor("Wk", [H, H], f16, kind="ExternalInput").ap(),
        "WvT": nc.dram_tensor("WvT", [H, H], f16, kind="ExternalInput").ap(),
        "bq_p": nc.dram_tensor("bq_p", [P, HT], f32, kind="ExternalInput").ap(),
        "bv_b": nc.dram_tensor("bv_b", [P, H], f16, kind="ExternalInput").ap(),
    }
    outs = {
        "self_out": nc.dram_tensor("self_out", [BPC, L, H], f16,
                                   kind="ExternalOutput").ap(),
        "opp_out": nc.dram_tensor("opp_out", [BPC, L, H], f16,
                                  kind="ExternalOutput").ap(),
    }
    with tile.TileContext(nc) as tc:
        with contextlib.ExitStack() as ctx:
            _build_core_kernel(ctx, tc, ins, outs)
    nc.compile()
    _NC_CACHE = nc
    return nc


def kernel(q, opp, Wq, bq, Wk, bk, Wv, bv):
    f16 = np.float16
    qT = np.ascontiguousarray(
        np.asarray(q, dtype=np.float32).astype(f16).transpose(0, 2, 1))
    oppT = np.ascontiguousarray(
        np.asarray(opp, dtype=np.float32).astype(f16).transpose(0, 2, 1))
    Wq = np.asarray(Wq, dtype=np.float32)
    Wk = np.asarray(Wk, dtype=np.float32)
    Wv = np.asarray(Wv, dtype=np.float32)
    bq = np.asarray(bq, dtype=np.float32)
    bv = np.asarray(bv, dtype=np.float32)
    # bk is mathematically irrelevant (softmax shift-invariance); unused.

    shared = {
        "WqT": np.ascontiguousarray(Wq.T.astype(f16)),
        "Wk": np.ascontiguousarray(Wk.astype(f16)),
        "WvT": np.ascontiguousarray(Wv.T.astype(f16)),
        "bq_p": np.ascontiguousarray(bq.reshape(HT, P).T),
        "bv_b": np.ascontiguousarray(np.tile(bv, (P, 1)).astype(f16)),
    }
    in_maps = []
    for c in range(NCORES):
        sl = slice(c * BPC, (c + 1) * BPC)
        in_maps.append({
            "qT": np.ascontiguousarray(qT[sl]),
            "oppT": np.ascontiguousarray(oppT[sl]),
            **shared,
        })

    nc = _get_module()
    res = run_bass_kernel_spmd(nc, in_maps, core_ids=list(range(NCORES)))
    self_out = np.concatenate(
        [r["self_out"].astype(np.float32) for r in res.results], axis=0)
    opp_out = np.concatenate(
        [r["opp_out"].astype(np.float32) for r in res.results], axis=0)
    return (self_out, opp_out)
